# revision 11
# baseline (speedup 1.0000x reference)
"""Trainium2 Bass kernel for nn_CNNNer (sparse band biaffine NER scorer).

Math collapse (everything after the GELU stage is linear):
  head = gelu(state@Wh+bh) ++ [1]          (features i = 0..200, i=200 is the 1)
  tail = gelu(state@Wt+bt) ++ [1]
  band[n,r,k] = head[n]^T U''_k tail[m],  m = n+r-64
      with U''_k = U_k + e_200 Wtp[k,:] + Whp[k,:]^T e_200^T
  scores'[n,r,t] = head_masked[n]^T UW_t tail_masked[m],
      UW_t = sum_k Wd[k,t] U''_k            (precomputed on host, [9,201,201])
  scores = scores' + bd  (host), masked-out entries = bd exactly.

Device work per core (8 cores; core = (batch b, query quarter), 256 queries,
384-position tail window). All IO in bf16 (tolerance is 2e-2):
  1. headT/tailT = gelu MLPs computed transposed ([feature, position]).
  2. step A: Uh_t[j, x] = sum_i UW[t,i,j] headT[i,x]          (9 tags)
  3. step B (tail-stationary, 6 weight loads total):
     S_t[m, x] = sum_j tailT[j, m] Uh_t[j, x] per 128-wide window chunk h
     paired with the query chunk(s) needing it: (h,xc) = (0,0),(1,0),(1,1),
     (2,1).  Band diagonals are extracted on host from the [m,t,x] blocks.

Perf structure (from trace analysis of prior versions):
  - Weights (Wh/Wt/bias/UW) are baked into the NEFF as Const tensors
    (inline_tensor), so only state-window + mask are staged per run.
    The build is cached keyed on the weight bytes; different weights
    just trigger a (seconds-long) rebuild, not wrong answers.
  - One dma_start chain can end up served by a single DMA engine
    (~20 GB/s), so sizable transfers are split into multiple chains with
    768-1536B per-partition-contiguous descriptors.
  - The scalar (Activation) queue issues NO DMAs: DIRECT2D descriptor
    generation executes on the issuing sequencer and would block the
    GELU act-table load + activations behind it.  Loads and writebacks
    go on sync (HWDGE) + gpsimd (SWDGE) queues only.
  - A short burst of junk matmuls at kernel start ramps the PE out of
    its low/mid p-state (2x clock) while the input DMAs are in flight.
"""

import hashlib
import os

import numpy as np

B, N, HID = 2, 1024, 768
BSZ = 200
W = 64
TAGS = 9
F = BSZ + 1  # 201 features incl the ones column
NQ = 256  # queries per core
NW = NQ + 2 * W  # 384 window positions per core
R = 2 * W + 1  # 129 band offsets
NCORES = 8
I2 = F - 128  # 73: second feature tile rows (i = 128..200)
F2 = BSZ - 128  # 72: second MLP output tile rows

_cache: dict = {}


def _build_nc(consts):
    import concourse.mybir as mybir
    import concourse.tile as tile
    from concourse import bacc

    dt = mybir.dt
    f32 = dt.float32
    bf16 = dt.bfloat16

    nc = bacc.Bacc(
        "TRN2", target_bir_lowering=False, debug=False, enable_asserts=False
    )
    # Per-run inputs (per-core): state window + key/query validity mask.
    xTd = nc.dram_tensor("xTd", [128, 6, NW], bf16, kind="ExternalInput").ap()
    mskd = nc.dram_tensor("mskd", [128, NW], bf16, kind="ExternalInput").ap()
    # Weights, baked into the NEFF (loaded to HBM at model-load time).
    whd = nc.inline_tensor(consts["whd"], name="whd").ap()
    wtd = nc.inline_tensor(consts["wtd"], name="wtd").ap()
    bias4 = nc.inline_tensor(consts["bias4"], name="bias4").ap()
    uw1d = nc.inline_tensor(consts["uw1"], name="uw1d").ap()
    uw2d = nc.inline_tensor(consts["uw2"], name="uw2d").ap()
    # output: 4 window-chunk/query-chunk combos of [m, t, x]
    sout = nc.dram_tensor("sout", [4, 128, TAGS, 128], bf16, kind="ExternalOutput").ap()

    gelu = {
        "gelu": mybir.ActivationFunctionType.Gelu,
        "identity": mybir.ActivationFunctionType.Identity,
    }[os.environ.get("BASSK_ACT", "gelu")]

    with tile.TileContext(nc) as tc:
        with tc.tile_pool(name="sb", bufs=1) as sb:
            # ---- SBUF tiles (split finely so loads unlock compute ASAP) ----
            x_sb = [sb.tile([128, 2, NW], bf16, name=f"x{i}") for i in range(3)]
            wh_sb = [sb.tile([128, 3, BSZ], bf16, name=f"wh{i}") for i in range(2)]
            wt_sb = [sb.tile([128, 3, BSZ], bf16, name=f"wt{i}") for i in range(2)]
            b_sb = sb.tile([128, 4], f32)
            m_sb = sb.tile([128, NW], bf16)
            uw1 = [sb.tile([128, 3, F], bf16, name=f"uw1g{g}") for g in range(3)]
            uw2 = [sb.tile([I2, 3, F], bf16, name=f"uw2g{g}") for g in range(3)]
            headT1 = sb.tile([128, NQ], bf16)
            headT2 = sb.tile([I2, NQ], bf16)
            tailT1 = sb.tile([128, NW], bf16)
            tailT2 = sb.tile([I2, NW], bf16)
            uh1 = sb.tile([128, TAGS, NQ], bf16)
            uh2 = sb.tile([I2, TAGS, NQ], bf16)
            junk = sb.tile([128, 512], bf16)
            s_cg = [
                [sb.tile([128, 3, 128], bf16, name=f"s{c}g{g}") for g in range(3)]
                for c in range(4)
            ]

            # ---- loads: sync(HWDGE) + gpsimd(SWDGE); scalar stays clean ----
            # junk memset first so the PE warmup can start ASAP
            nc.gpsimd.memset(junk, 0.0)
            nc.sync.dma_start(out=x_sb[0], in_=xTd[:, 0:2, :])
            nc.gpsimd.dma_start(out=b_sb, in_=bias4)
            nc.sync.dma_start(out=wh_sb[0], in_=whd[:, 0:3, :])
            nc.gpsimd.dma_start(out=m_sb[0:64, :], in_=mskd[0:64, :])
            nc.sync.dma_start(out=x_sb[1], in_=xTd[:, 2:4, :])
            nc.gpsimd.dma_start(out=m_sb[64:128, :], in_=mskd[64:128, :])
            nc.sync.dma_start(out=x_sb[2], in_=xTd[:, 4:6, :])
            # masked ones-feature rows (engines can't address partition 72)
            nc.gpsimd.dma_start(
                out=headT2[F2 : F2 + 1, :], in_=mskd[0:1, W : W + NQ]
            )
            nc.sync.dma_start(out=wh_sb[1], in_=whd[:, 3:6, :])
            nc.gpsimd.dma_start(out=tailT2[F2 : F2 + 1, :], in_=mskd[0:1, 0:NW])
            nc.sync.dma_start(out=uw1[0], in_=uw1d[:, 0:3, :])
            nc.gpsimd.dma_start(out=wt_sb[0], in_=wtd[:, 0:3, :])
            nc.sync.dma_start(out=uw1[1], in_=uw1d[:, 3:6, :])
            nc.gpsimd.dma_start(out=wt_sb[1], in_=wtd[:, 3:6, :])
            nc.sync.dma_start(out=uw1[2], in_=uw1d[:, 6:9, :])
            nc.gpsimd.dma_start(out=uw2[0], in_=uw2d[:, 0:3, :])
            nc.gpsimd.dma_start(out=uw2[1], in_=uw2d[:, 3:6, :])
            nc.gpsimd.dma_start(out=uw2[2], in_=uw2d[:, 6:9, :])

            # ---- PE p-state warmup while DMAs land ----
            with tc.tile_pool(name="psj", bufs=1, space="PSUM") as psj:
                pj = psj.tile([128, 512], f32, tag="jk")
                for _ in range(6):
                    nc.tensor.matmul(
                        pj, junk[:, 0:128], junk, start=True, stop=True
                    )

            # ---- MLPs: o = gelu(W^T x + b), computed transposed ----
            bh1, bt1 = b_sb[:, 0:1], b_sb[:, 1:2]
            bh2, bt2 = b_sb[0:F2, 2:3], b_sb[0:F2, 3:4]
            with tc.tile_pool(name="psm", bufs=2, space="PSUM") as psm:
                for w_t, b1, b2, o1, o2, c0, ncols in (
                    (wh_sb, bh1, bh2, headT1, headT2, W, NQ),
                    (wt_sb, bt1, bt2, tailT1, tailT2, 0, NW),
                ):
                    for fw, f0, o, bias in ((128, 0, o1, b1), (F2, 128, o2, b2)):
                        pm = psm.tile([fw, ncols], f32, tag="pm")
                        for ht in range(6):
                            nc.tensor.matmul(
                                pm,
                                w_t[ht // 3][:, ht % 3, f0 : f0 + fw],
                                x_sb[ht // 2][:, ht % 2, c0 : c0 + ncols],
                                start=(ht == 0),
                                stop=(ht == 5),
                            )
                        nc.scalar.activation(
                            out=o[0:fw, :], in_=pm, func=gelu, bias=bias
                        )
                    nc.vector.tensor_mul(o1, o1, m_sb[0:128, c0 : c0 + ncols])
                    nc.vector.tensor_mul(
                        o2[0:F2, :], o2[0:F2, :], m_sb[0:F2, c0 : c0 + ncols]
                    )

                # ---- step A: Uh_t[j, x] = sum_i UW[t,i,j] headT[i,x] ----
                # psa nests inside psm so it gets fresh PSUM banks — the
                # first A matmuls must not WAR-wait on the MLP gelu reads
                with tc.tile_pool(name="psa", bufs=4, space="PSUM") as psa:
                    for t in range(TAGS):
                        g, tl = divmod(t, 3)
                        for jw, j0, uh in ((128, 0, uh1), (I2, 128, uh2)):
                            pa = psa.tile([jw, NQ], f32, tag="pa")
                            nc.tensor.matmul(
                                pa,
                                uw1[g][:, tl, j0 : j0 + jw],
                                headT1,
                                start=True,
                                stop=False,
                            )
                            nc.tensor.matmul(
                                pa,
                                uw2[g][:, tl, j0 : j0 + jw],
                                headT2,
                                start=False,
                                stop=True,
                            )
                            nc.any.tensor_copy(uh[:, t, :], pa)

            # ---- step B: S[m, t, x] = sum_j tailT[j, m] Uh_t[j, x] ----
            # combos: (window chunk h, query chunk xc)
            combos = {0: ((0, 0),), 1: ((1, 0), (2, 1)), 2: ((3, 1),)}
            wbq = (nc.sync, nc.gpsimd, nc.scalar)
            wbi = 0
            with tc.tile_pool(name="psb", bufs=6, space="PSUM") as psb:
                pb: dict = {}
                for h in range(3):
                    for jt, (tl_t, uh_t) in enumerate(
                        ((tailT1, uh1), (tailT2, uh2))
                    ):
                        for c, xc in combos[h]:
                            for g in range(3):
                                if jt == 0:
                                    pb[c, g] = psb.tile(
                                        [128, 3, 128],
                                        f32,
                                        tag="pb",
                                        name=f"pb{c}_{g}",
                                    )
                                nc.tensor.matmul(
                                    pb[c, g],
                                    tl_t[:, 128 * h : 128 * h + 128],
                                    uh_t[
                                        :,
                                        3 * g : 3 * g + 3,
                                        128 * xc : 128 * xc + 128,
                                    ],
                                    start=(jt == 0),
                                    stop=(jt == 1),
                                )
                    for c, xc in combos[h]:
                        for g in range(3):
                            nc.any.tensor_copy(s_cg[c][g], pb[c, g])
                            wbq[wbi % 3].dma_start(
                                out=sout[c, :, 3 * g : 3 * g + 3, :],
                                in_=s_cg[c][g],
                            )
                            wbi += 1

    nc.compile()
    return nc


def _prep_consts(Wh, bh, Wt, bt, U, Wcat, Wd):
    """Fold U/Wcat/Wd into UW[9,201,201]; arrange weights for the device."""
    import ml_dtypes

    bf16 = ml_dtypes.bfloat16

    Whp = Wcat[:, :F]  # [K, 201]
    Wtp = Wcat[:, F:]  # [K, 201]
    U2 = U.astype(np.float64).copy()
    U2[:, F - 1, :] += Wtp  # head ones-row picks up the tail term
    U2[:, :, F - 1] += Whp  # tail ones-col picks up the head term
    UW = np.einsum("kt,kij->tij", Wd.astype(np.float64), U2).astype(np.float32)
    UWi = np.ascontiguousarray(UW.transpose(1, 0, 2))  # [i, t, j]

    def tr6(w):  # [768, m] -> [128, 6, m] partition-major
        m = w.shape[1]
        return np.ascontiguousarray(
            w.reshape(6, 128, m).transpose(1, 0, 2)
        ).astype(bf16)

    return {
        "whd": tr6(Wh),
        "wtd": tr6(Wt),
        "bias4": np.ascontiguousarray(
            np.stack(
                [
                    bh[0:128],
                    bt[0:128],
                    np.pad(bh[128:BSZ], (0, 128 - F2)),
                    np.pad(bt[128:BSZ], (0, 128 - F2)),
                ],
                axis=1,
            ).astype(np.float32)
        ),
        "uw1": np.ascontiguousarray(UWi[0:128]).astype(bf16),
        "uw2": np.ascontiguousarray(UWi[128:F]).astype(bf16),
    }


def _get_nc(consts):
    key = hashlib.md5(
        b"".join(np.ascontiguousarray(v).tobytes() for v in consts.values())
    ).hexdigest()
    if _cache.get("nc_key") != key:
        _cache["nc"] = _build_nc(consts)
        _cache["nc_key"] = key
    return _cache["nc"]


def _install_ntff_hook():
    """Profiling-only (BASSK_TRACE=1): provide antenv.axon_hooks if the
    image lacks it, wired to the libaxon NTFF capture via ctypes."""
    import sys
    import types

    try:
        from antenv.axon_hooks import get_axon_ntff_profile_hook  # noqa: F401

        return
    except ImportError:
        pass
    from trn_agent_boot.trn_boot import _ntff_profile_via_ctypes

    hook = _ntff_profile_via_ctypes("/opt/axon/libaxon_pjrt.so")
    mod = types.ModuleType("antenv.axon_hooks")
    mod._hook = hook
    mod.get_axon_ntff_profile_hook = lambda: mod._hook
    mod.set_axon_ntff_profile_hook = lambda h: setattr(mod, "_hook", h)
    sys.modules["antenv.axon_hooks"] = mod


def _host_prep(state, lengths):
    """Per-core inputs: transposed state window + validity mask."""
    import ml_dtypes

    bf16 = ml_dtypes.bfloat16

    in_maps = []
    for b in range(B):
        for qi in range(N // NQ):
            q0 = qi * NQ
            lo = q0 - W
            xw = np.zeros((NW, HID), np.float32)
            s, e = max(lo, 0), min(q0 + NQ + W, N)
            xw[s - lo : e - lo] = state[b, s:e]
            pos = lo + np.arange(NW)
            mrow = ((pos >= 0) & (pos < N) & (pos < lengths[b])).astype(
                np.float32
            )
            xT = np.ascontiguousarray(xw.T)  # [768, 384]
            in_maps.append(
                {
                    "xTd": np.ascontiguousarray(
                        xT.reshape(6, 128, NW).transpose(1, 0, 2)
                    ).astype(bf16),
                    "mskd": np.ascontiguousarray(
                        np.broadcast_to(mrow[None, :], (128, NW))
                    ).astype(bf16),
                }
            )
    return in_maps


def _assemble(outs, bd):
    """outs: NCORES arrays [4, 128, TAGS, 128] -> scores [B, N, R, TAGS]."""
    scores = np.empty((B, N, R, TAGS), np.float32)
    widx = np.arange(128)[:, None] + np.arange(R)[None, :]  # [128, 129]
    xidx = np.arange(128)[:, None]
    for c, S in enumerate(outs):
        S = np.asarray(S, dtype=np.float32)  # upcast from bf16
        b, qi = divmod(c, N // NQ)
        for qc in range(2):
            # window blocks covering query chunk qc: [256 w, TAGS, 128 x]
            arr = np.concatenate([S[2 * qc], S[2 * qc + 1]], axis=0)
            g = arr[widx, :, xidx]  # [128, 129, TAGS]
            q0 = qi * NQ + qc * 128
            scores[b, q0 : q0 + 128] = g
    scores += bd.astype(np.float32)[None, None, None, :]
    return np.where(np.isfinite(scores), scores, 0.0).astype(np.float32)


def kernel(**inputs):
    state = np.asarray(inputs["state"], np.float32)
    lengths = np.asarray(inputs["lengths"]).astype(np.int64)
    Wh = np.ascontiguousarray(np.asarray(inputs["Wh"], np.float32))
    bh = np.asarray(inputs["bh"], np.float32)
    Wt = np.ascontiguousarray(np.asarray(inputs["Wt"], np.float32))
    bt = np.asarray(inputs["bt"], np.float32)
    U = np.asarray(inputs["U"], np.float32)
    Wcat = np.asarray(inputs["Wcat"], np.float32)
    Wd = np.asarray(inputs["Wd"], np.float32)
    bd = np.asarray(inputs["bd"], np.float32)

    consts = _prep_consts(Wh, bh, Wt, bt, U, Wcat, Wd)
    in_maps = _host_prep(state, lengths)
    nc = _get_nc(consts)

    if os.environ.get("BASSK_SIM"):
        from concourse.bass_interp import CoreSim

        outs = []
        for im in in_maps:
            sim = CoreSim(nc, trace=False)
            for k, v in im.items():
                sim.tensor(k)[:] = v
            sim.simulate()
            outs.append(sim.tensor("sout").copy())
    else:
        trace = bool(os.environ.get("BASSK_TRACE"))
        if trace:
            _install_ntff_hook()
        from concourse.bass_utils import run_bass_kernel_spmd

        try:
            res = run_bass_kernel_spmd(
                nc, in_maps, core_ids=list(range(NCORES)), trace=trace
            )
        except Exception:
            # transient NRT/device hiccups recover on a fresh attempt
            import time

            time.sleep(2.0)
            res = run_bass_kernel_spmd(
                nc, in_maps, core_ids=list(range(NCORES)), trace=trace
            )
        _cache["last_result"] = res
        outs = [r["sout"] for r in res.results]

    return _assemble(outs, bd)


# revision 13
# speedup vs baseline: 1.1056x; 1.1056x over previous
"""Trainium2 Bass kernel for nn_CNNNer (sparse band biaffine NER scorer).

Math collapse (everything after the GELU stage is linear):
  head = gelu(state@Wh+bh) ++ [1]          (features i = 0..200, i=200 is the 1)
  tail = gelu(state@Wt+bt) ++ [1]
  band[n,r,k] = head[n]^T U''_k tail[m],  m = n+r-64
      with U''_k = U_k + e_200 Wtp[k,:] + Whp[k,:]^T e_200^T
  scores'[n,r,t] = head_masked[n]^T UW_t tail_masked[m],
      UW_t = sum_k Wd[k,t] U''_k            (precomputed on host, [9,201,201])
  scores = scores' + bd  (host), masked-out entries = bd exactly.

Device work per core (8 cores; core = (batch b, query quarter), 256 queries,
384-position tail window). All IO in bf16 (tolerance is 2e-2):
  1. headT/tailT = gelu MLPs computed transposed ([feature, position]).
  2. step A: Uh_t[j, x] = sum_i UW[t,i,j] headT[i,x]          (9 tags)
  3. step B (tail-stationary, 6 weight loads total):
     S_t[m, x] = sum_j tailT[j, m] Uh_t[j, x] per 128-wide window chunk h
     paired with the query chunk(s) needing it: (h,xc) = (0,0),(1,0),(1,1),
     (2,1).  Band diagonals are extracted on host from the [m,t,x] blocks.

Perf structure (from trace analysis of prior versions):
  - Weights (Wh/Wt/bias/UW) are baked into the NEFF as Const tensors
    (inline_tensor), so only state-window + mask are staged per run.
    The build is cached keyed on the weight bytes; different weights
    just trigger a (seconds-long) rebuild, not wrong answers.
  - One dma_start chain can end up served by a single DMA engine
    (~20 GB/s), so sizable transfers are split into multiple chains with
    768-1536B per-partition-contiguous descriptors.
  - The scalar (Activation) queue issues NO DMAs: DIRECT2D descriptor
    generation executes on the issuing sequencer and would block the
    GELU act-table load + activations behind it.  Loads and writebacks
    go on sync (HWDGE) + gpsimd (SWDGE) queues only.
  - A short burst of junk matmuls at kernel start ramps the PE out of
    its low/mid p-state (2x clock) while the input DMAs are in flight.
"""

import hashlib
import os

import numpy as np

B, N, HID = 2, 1024, 768
BSZ = 200
W = 64
TAGS = 9
F = BSZ + 1  # 201 features incl the ones column
NQ = 256  # queries per core
NW = NQ + 2 * W  # 384 window positions per core
R = 2 * W + 1  # 129 band offsets
NCORES = 8
I2 = F - 128  # 73: second feature tile rows (i = 128..200)
F2 = BSZ - 128  # 72: second MLP output tile rows

_cache: dict = {}


def _build_nc(consts):
    import concourse.mybir as mybir
    import concourse.tile as tile
    from concourse import bacc

    dt = mybir.dt
    f32 = dt.float32
    bf16 = dt.bfloat16

    nc = bacc.Bacc(
        "TRN2", target_bir_lowering=False, debug=False, enable_asserts=False
    )
    # Per-run inputs (per-core): state window + key/query validity mask.
    xTd = nc.dram_tensor("xTd", [128, 6, NW], bf16, kind="ExternalInput").ap()
    mskd = nc.dram_tensor("mskd", [128, NW], bf16, kind="ExternalInput").ap()
    # Weights, baked into the NEFF (loaded to HBM at model-load time).
    whd = nc.inline_tensor(consts["whd"], name="whd").ap()
    wtd = nc.inline_tensor(consts["wtd"], name="wtd").ap()
    bias4 = nc.inline_tensor(consts["bias4"], name="bias4").ap()
    uw1d = nc.inline_tensor(consts["uw1"], name="uw1d").ap()
    uw2d = nc.inline_tensor(consts["uw2"], name="uw2d").ap()
    # output: 4 window-chunk/query-chunk combos of [m, t, x]
    sout = nc.dram_tensor("sout", [4, 128, TAGS, 128], bf16, kind="ExternalOutput").ap()

    gelu = {
        "gelu": mybir.ActivationFunctionType.Gelu,
        "identity": mybir.ActivationFunctionType.Identity,
    }[os.environ.get("BASSK_ACT", "gelu")]

    with tile.TileContext(nc) as tc:
        with tc.tile_pool(name="sb", bufs=1) as sb:
            # ---- SBUF tiles (split finely so loads unlock compute ASAP) ----
            x_sb = [sb.tile([128, 2, NW], bf16, name=f"x{i}") for i in range(3)]
            wh_sb = [sb.tile([128, 3, BSZ], bf16, name=f"wh{i}") for i in range(2)]
            wt_sb = [sb.tile([128, 3, BSZ], bf16, name=f"wt{i}") for i in range(2)]
            b_sb = sb.tile([128, 4], f32)
            m_sb = sb.tile([128, NW], bf16)
            uw1 = [sb.tile([128, 3, F], bf16, name=f"uw1g{g}") for g in range(3)]
            uw2 = [sb.tile([I2, 3, F], bf16, name=f"uw2g{g}") for g in range(3)]
            headT1 = sb.tile([128, NQ], bf16)
            headT2 = sb.tile([I2, NQ], bf16)
            tailT1 = sb.tile([128, NW], bf16)
            tailT2 = sb.tile([I2, NW], bf16)
            uh1 = sb.tile([128, TAGS, NQ], bf16)
            uh2 = sb.tile([I2, TAGS, NQ], bf16)
            junk = sb.tile([128, 512], bf16)
            s_cg = [
                [sb.tile([128, 3, 128], bf16, name=f"s{c}g{g}") for g in range(3)]
                for c in range(4)
            ]

            # ---- loads: sync(HWDGE) + gpsimd(SWDGE); scalar stays clean ----
            # junk memset first so the PE warmup can start ASAP
            nc.gpsimd.memset(junk, 0.0)
            nc.sync.dma_start(out=x_sb[0], in_=xTd[:, 0:2, :])
            nc.gpsimd.dma_start(out=b_sb, in_=bias4)
            nc.sync.dma_start(out=wh_sb[0], in_=whd[:, 0:3, :])
            nc.gpsimd.dma_start(out=wh_sb[1], in_=whd[:, 3:6, :])
            nc.sync.dma_start(out=x_sb[1], in_=xTd[:, 2:4, :])
            nc.gpsimd.dma_start(out=m_sb[0:64, :], in_=mskd[0:64, :])
            nc.sync.dma_start(out=x_sb[2], in_=xTd[:, 4:6, :])
            nc.gpsimd.dma_start(out=m_sb[64:128, :], in_=mskd[64:128, :])
            # masked ones-feature rows (engines can't address partition 72)
            nc.gpsimd.dma_start(
                out=headT2[F2 : F2 + 1, :], in_=mskd[0:1, W : W + NQ]
            )
            nc.gpsimd.dma_start(out=tailT2[F2 : F2 + 1, :], in_=mskd[0:1, 0:NW])
            nc.sync.dma_start(out=uw1[0], in_=uw1d[:, 0:3, :])
            nc.gpsimd.dma_start(out=wt_sb[0], in_=wtd[:, 0:3, :])
            nc.sync.dma_start(out=uw1[1], in_=uw1d[:, 3:6, :])
            nc.gpsimd.dma_start(out=wt_sb[1], in_=wtd[:, 3:6, :])
            nc.sync.dma_start(out=uw1[2], in_=uw1d[:, 6:9, :])
            nc.gpsimd.dma_start(out=uw2[0], in_=uw2d[:, 0:3, :])
            nc.gpsimd.dma_start(out=uw2[1], in_=uw2d[:, 3:6, :])
            nc.gpsimd.dma_start(out=uw2[2], in_=uw2d[:, 6:9, :])

            # ---- PE p-state warmup while DMAs land ----
            with tc.tile_pool(name="psj", bufs=1, space="PSUM") as psj:
                pj = psj.tile([128, 512], f32, tag="jk")
                for _ in range(6):
                    nc.tensor.matmul(
                        pj, junk[:, 0:128], junk, start=True, stop=True
                    )

            # ---- MLPs: o = gelu(W^T x + b), computed transposed ----
            bh1, bt1 = b_sb[:, 0:1], b_sb[:, 1:2]
            bh2, bt2 = b_sb[0:F2, 2:3], b_sb[0:F2, 3:4]
            with tc.tile_pool(name="psm", bufs=4, space="PSUM") as psm:
                for w_t, b1, b2, o1, o2, c0, ncols in (
                    (wh_sb, bh1, bh2, headT1, headT2, W, NQ),
                    (wt_sb, bt1, bt2, tailT1, tailT2, 0, NW),
                ):
                    for fw, f0, o, bias in ((128, 0, o1, b1), (F2, 128, o2, b2)):
                        pm = psm.tile([fw, ncols], f32, tag="pm")
                        for ht in range(6):
                            nc.tensor.matmul(
                                pm,
                                w_t[ht // 3][:, ht % 3, f0 : f0 + fw],
                                x_sb[ht // 2][:, ht % 2, c0 : c0 + ncols],
                                start=(ht == 0),
                                stop=(ht == 5),
                            )
                        nc.scalar.activation(
                            out=o[0:fw, :], in_=pm, func=gelu, bias=bias
                        )
                    nc.vector.tensor_mul(o1, o1, m_sb[0:128, c0 : c0 + ncols])
                    nc.vector.tensor_mul(
                        o2[0:F2, :], o2[0:F2, :], m_sb[0:F2, c0 : c0 + ncols]
                    )

                # ---- step A: Uh_t[j, x] = sum_i UW[t,i,j] headT[i,x] ----
                # psa nests inside psm so it gets fresh PSUM banks — the
                # first A matmuls must not WAR-wait on the MLP gelu reads
                with tc.tile_pool(name="psa", bufs=4, space="PSUM") as psa:
                    for t in range(TAGS):
                        g, tl = divmod(t, 3)
                        for jw, j0, uh in ((128, 0, uh1), (I2, 128, uh2)):
                            pa = psa.tile([jw, NQ], f32, tag="pa")
                            nc.tensor.matmul(
                                pa,
                                uw1[g][:, tl, j0 : j0 + jw],
                                headT1,
                                start=True,
                                stop=False,
                            )
                            nc.tensor.matmul(
                                pa,
                                uw2[g][:, tl, j0 : j0 + jw],
                                headT2,
                                start=False,
                                stop=True,
                            )
                            nc.any.tensor_copy(uh[:, t, :], pa)

            # ---- step B: S[m, t, x] = sum_j tailT[j, m] Uh_t[j, x] ----
            # combos: (window chunk h, query chunk xc)
            combos = {0: ((0, 0),), 1: ((1, 0), (2, 1)), 2: ((3, 1),)}
            wbq = (nc.sync, nc.gpsimd, nc.scalar)
            wbi = 0
            with tc.tile_pool(name="psb", bufs=6, space="PSUM") as psb:
                pb: dict = {}
                for h in range(3):
                    for jt, (tl_t, uh_t) in enumerate(
                        ((tailT1, uh1), (tailT2, uh2))
                    ):
                        for c, xc in combos[h]:
                            for g in range(3):
                                if jt == 0:
                                    pb[c, g] = psb.tile(
                                        [128, 3, 128],
                                        f32,
                                        tag="pb",
                                        name=f"pb{c}_{g}",
                                    )
                                nc.tensor.matmul(
                                    pb[c, g],
                                    tl_t[:, 128 * h : 128 * h + 128],
                                    uh_t[
                                        :,
                                        3 * g : 3 * g + 3,
                                        128 * xc : 128 * xc + 128,
                                    ],
                                    start=(jt == 0),
                                    stop=(jt == 1),
                                )
                    for c, xc in combos[h]:
                        for g in range(3):
                            nc.any.tensor_copy(s_cg[c][g], pb[c, g])
                            wbq[wbi % 3].dma_start(
                                out=sout[c, :, 3 * g : 3 * g + 3, :],
                                in_=s_cg[c][g],
                            )
                            wbi += 1

    nc.compile()
    return nc


def _prep_consts(Wh, bh, Wt, bt, U, Wcat, Wd):
    """Fold U/Wcat/Wd into UW[9,201,201]; arrange weights for the device."""
    import ml_dtypes

    bf16 = ml_dtypes.bfloat16

    Whp = Wcat[:, :F]  # [K, 201]
    Wtp = Wcat[:, F:]  # [K, 201]
    U2 = U.astype(np.float64).copy()
    U2[:, F - 1, :] += Wtp  # head ones-row picks up the tail term
    U2[:, :, F - 1] += Whp  # tail ones-col picks up the head term
    UW = np.einsum("kt,kij->tij", Wd.astype(np.float64), U2).astype(np.float32)
    UWi = np.ascontiguousarray(UW.transpose(1, 0, 2))  # [i, t, j]

    def tr6(w):  # [768, m] -> [128, 6, m] partition-major
        m = w.shape[1]
        return np.ascontiguousarray(
            w.reshape(6, 128, m).transpose(1, 0, 2)
        ).astype(bf16)

    return {
        "whd": tr6(Wh),
        "wtd": tr6(Wt),
        "bias4": np.ascontiguousarray(
            np.stack(
                [
                    bh[0:128],
                    bt[0:128],
                    np.pad(bh[128:BSZ], (0, 128 - F2)),
                    np.pad(bt[128:BSZ], (0, 128 - F2)),
                ],
                axis=1,
            ).astype(np.float32)
        ),
        "uw1": np.ascontiguousarray(UWi[0:128]).astype(bf16),
        "uw2": np.ascontiguousarray(UWi[128:F]).astype(bf16),
    }


def _get_nc(consts):
    key = hashlib.md5(
        b"".join(np.ascontiguousarray(v).tobytes() for v in consts.values())
    ).hexdigest()
    if _cache.get("nc_key") != key:
        _cache["nc"] = _build_nc(consts)
        _cache["nc_key"] = key
    return _cache["nc"]


def _install_ntff_hook():
    """Profiling-only (BASSK_TRACE=1): provide antenv.axon_hooks if the
    image lacks it, wired to the libaxon NTFF capture via ctypes."""
    import sys
    import types

    try:
        from antenv.axon_hooks import get_axon_ntff_profile_hook  # noqa: F401

        return
    except ImportError:
        pass
    from trn_agent_boot.trn_boot import _ntff_profile_via_ctypes

    hook = _ntff_profile_via_ctypes("/opt/axon/libaxon_pjrt.so")
    mod = types.ModuleType("antenv.axon_hooks")
    mod._hook = hook
    mod.get_axon_ntff_profile_hook = lambda: mod._hook
    mod.set_axon_ntff_profile_hook = lambda h: setattr(mod, "_hook", h)
    sys.modules["antenv.axon_hooks"] = mod


def _host_prep(state, lengths):
    """Per-core inputs: transposed state window + validity mask."""
    import ml_dtypes

    bf16 = ml_dtypes.bfloat16

    in_maps = []
    for b in range(B):
        for qi in range(N // NQ):
            q0 = qi * NQ
            lo = q0 - W
            xw = np.zeros((NW, HID), np.float32)
            s, e = max(lo, 0), min(q0 + NQ + W, N)
            xw[s - lo : e - lo] = state[b, s:e]
            pos = lo + np.arange(NW)
            mrow = ((pos >= 0) & (pos < N) & (pos < lengths[b])).astype(
                np.float32
            )
            xT = np.ascontiguousarray(xw.T)  # [768, 384]
            in_maps.append(
                {
                    "xTd": np.ascontiguousarray(
                        xT.reshape(6, 128, NW).transpose(1, 0, 2)
                    ).astype(bf16),
                    "mskd": np.ascontiguousarray(
                        np.broadcast_to(mrow[None, :], (128, NW))
                    ).astype(bf16),
                }
            )
    return in_maps


def _assemble(outs, bd):
    """outs: NCORES arrays [4, 128, TAGS, 128] -> scores [B, N, R, TAGS]."""
    scores = np.empty((B, N, R, TAGS), np.float32)
    widx = np.arange(128)[:, None] + np.arange(R)[None, :]  # [128, 129]
    xidx = np.arange(128)[:, None]
    for c, S in enumerate(outs):
        S = np.asarray(S, dtype=np.float32)  # upcast from bf16
        b, qi = divmod(c, N // NQ)
        for qc in range(2):
            # window blocks covering query chunk qc: [256 w, TAGS, 128 x]
            arr = np.concatenate([S[2 * qc], S[2 * qc + 1]], axis=0)
            g = arr[widx, :, xidx]  # [128, 129, TAGS]
            q0 = qi * NQ + qc * 128
            scores[b, q0 : q0 + 128] = g
    scores += bd.astype(np.float32)[None, None, None, :]
    return np.where(np.isfinite(scores), scores, 0.0).astype(np.float32)


def kernel(**inputs):
    state = np.asarray(inputs["state"], np.float32)
    lengths = np.asarray(inputs["lengths"]).astype(np.int64)
    Wh = np.ascontiguousarray(np.asarray(inputs["Wh"], np.float32))
    bh = np.asarray(inputs["bh"], np.float32)
    Wt = np.ascontiguousarray(np.asarray(inputs["Wt"], np.float32))
    bt = np.asarray(inputs["bt"], np.float32)
    U = np.asarray(inputs["U"], np.float32)
    Wcat = np.asarray(inputs["Wcat"], np.float32)
    Wd = np.asarray(inputs["Wd"], np.float32)
    bd = np.asarray(inputs["bd"], np.float32)

    consts = _prep_consts(Wh, bh, Wt, bt, U, Wcat, Wd)
    in_maps = _host_prep(state, lengths)
    nc = _get_nc(consts)

    if os.environ.get("BASSK_SIM"):
        from concourse.bass_interp import CoreSim

        outs = []
        for im in in_maps:
            sim = CoreSim(nc, trace=False)
            for k, v in im.items():
                sim.tensor(k)[:] = v
            sim.simulate()
            outs.append(sim.tensor("sout").copy())
    else:
        trace = bool(os.environ.get("BASSK_TRACE"))
        if trace:
            _install_ntff_hook()
        from concourse.bass_utils import run_bass_kernel_spmd

        try:
            res = run_bass_kernel_spmd(
                nc, in_maps, core_ids=list(range(NCORES)), trace=trace
            )
        except Exception:
            # transient NRT/device hiccups recover on a fresh attempt
            import time

            time.sleep(2.0)
            res = run_bass_kernel_spmd(
                nc, in_maps, core_ids=list(range(NCORES)), trace=trace
            )
        _cache["last_result"] = res
        outs = [r["sout"] for r in res.results]

    return _assemble(outs, bd)


# revision 14
# speedup vs baseline: 1.1213x; 1.0142x over previous
"""Trainium2 Bass kernel for nn_CNNNer (sparse band biaffine NER scorer).

Math collapse (everything after the GELU stage is linear):
  head = gelu(state@Wh+bh) ++ [1]          (features i = 0..200, i=200 is the 1)
  tail = gelu(state@Wt+bt) ++ [1]
  band[n,r,k] = head[n]^T U''_k tail[m],  m = n+r-64
      with U''_k = U_k + e_200 Wtp[k,:] + Whp[k,:]^T e_200^T
  scores'[n,r,t] = head_masked[n]^T UW_t tail_masked[m],
      UW_t = sum_k Wd[k,t] U''_k            (precomputed on host, [9,201,201])
  scores = scores' + bd  (host), masked-out entries = bd exactly.

Device work per core (8 cores; core = (batch b, query quarter), 256 queries,
384-position tail window). All IO in bf16 (tolerance is 2e-2):
  1. headT/tailT = gelu MLPs computed transposed ([feature, position]).
  2. step A: Uh_t[j, x] = sum_i UW[t,i,j] headT[i,x]          (9 tags)
  3. step B (tail-stationary, 6 weight loads total):
     S_t[m, x] = sum_j tailT[j, m] Uh_t[j, x] per 128-wide window chunk h
     paired with the query chunk(s) needing it: (h,xc) = (0,0),(1,0),(1,1),
     (2,1).  Band diagonals are extracted on host from the [m,t,x] blocks.

Perf structure (from trace analysis of prior versions):
  - Weights (Wh/Wt/bias/UW) are baked into the NEFF as Const tensors
    (inline_tensor), so only state-window + mask are staged per run.
    The build is cached keyed on the weight bytes; different weights
    just trigger a (seconds-long) rebuild, not wrong answers.
  - One dma_start chain can end up served by a single DMA engine
    (~20 GB/s), so sizable transfers are split into multiple chains with
    768-1536B per-partition-contiguous descriptors.
  - The scalar (Activation) queue issues NO DMAs: DIRECT2D descriptor
    generation executes on the issuing sequencer and would block the
    GELU act-table load + activations behind it.  Loads and writebacks
    go on sync (HWDGE) + gpsimd (SWDGE) queues only.
  - A short burst of junk matmuls at kernel start ramps the PE out of
    its low/mid p-state (2x clock) while the input DMAs are in flight.
"""

import hashlib
import os

import numpy as np

B, N, HID = 2, 1024, 768
BSZ = 200
W = 64
TAGS = 9
F = BSZ + 1  # 201 features incl the ones column
NQ = 256  # queries per core
NW = NQ + 2 * W  # 384 window positions per core
R = 2 * W + 1  # 129 band offsets
NCORES = 8
I2 = F - 128  # 73: second feature tile rows (i = 128..200)
F2 = BSZ - 128  # 72: second MLP output tile rows

_cache: dict = {}


def _build_nc(consts):
    import concourse.mybir as mybir
    import concourse.tile as tile
    from concourse import bacc

    dt = mybir.dt
    f32 = dt.float32
    bf16 = dt.bfloat16

    nc = bacc.Bacc(
        "TRN2", target_bir_lowering=False, debug=False, enable_asserts=False
    )
    # Per-run inputs (per-core): state window + key/query validity mask.
    xTd = nc.dram_tensor("xTd", [128, 6, NW], bf16, kind="ExternalInput").ap()
    mskd = nc.dram_tensor("mskd", [128, NW], bf16, kind="ExternalInput").ap()
    # Weights, baked into the NEFF (loaded to HBM at model-load time).
    whd = nc.inline_tensor(consts["whd"], name="whd").ap()
    wtd = nc.inline_tensor(consts["wtd"], name="wtd").ap()
    bias4 = nc.inline_tensor(consts["bias4"], name="bias4").ap()
    uw1d = nc.inline_tensor(consts["uw1"], name="uw1d").ap()
    uw2d = nc.inline_tensor(consts["uw2"], name="uw2d").ap()
    # output: 4 window-chunk/query-chunk combos of [m, t, x]
    sout = nc.dram_tensor("sout", [4, 128, TAGS, 128], bf16, kind="ExternalOutput").ap()

    gelu = {
        "gelu": mybir.ActivationFunctionType.Gelu,
        "identity": mybir.ActivationFunctionType.Identity,
    }[os.environ.get("BASSK_ACT", "gelu")]

    with tile.TileContext(nc) as tc:
        with tc.tile_pool(name="sb", bufs=1) as sb:
            # ---- SBUF tiles (split finely so loads unlock compute ASAP) ----
            x_sb = [sb.tile([128, 2, NW], bf16, name=f"x{i}") for i in range(3)]
            wh_sb = [sb.tile([128, 3, BSZ], bf16, name=f"wh{i}") for i in range(2)]
            wt_sb = [sb.tile([128, 3, BSZ], bf16, name=f"wt{i}") for i in range(2)]
            b_sb = sb.tile([128, 4], f32)
            m_sb = sb.tile([128, NW], bf16)
            uw1 = [sb.tile([128, 3, F], bf16, name=f"uw1g{g}") for g in range(3)]
            uw2 = [sb.tile([I2, 3, F], bf16, name=f"uw2g{g}") for g in range(3)]
            headT1 = sb.tile([128, NQ], bf16)
            headT2 = sb.tile([I2, NQ], bf16)
            tailT1 = sb.tile([128, NW], bf16)
            tailT2 = sb.tile([I2, NW], bf16)
            uh1 = sb.tile([128, TAGS, NQ], bf16)
            uh2 = sb.tile([I2, TAGS, NQ], bf16)
            junk = sb.tile([128, 512], bf16)
            s_cg = [
                [sb.tile([128, 3, 128], bf16, name=f"s{c}g{g}") for g in range(3)]
                for c in range(4)
            ]

            # ---- loads: sync(HWDGE) + gpsimd(SWDGE); scalar stays clean ----
            # junk memset first so the PE warmup can start ASAP
            nc.gpsimd.memset(junk, 0.0)
            nc.sync.dma_start(out=x_sb[0], in_=xTd[:, 0:2, :])
            nc.gpsimd.dma_start(out=b_sb, in_=bias4)
            nc.sync.dma_start(out=wh_sb[0], in_=whd[:, 0:3, :])
            nc.gpsimd.dma_start(out=m_sb[0:64, :], in_=mskd[0:64, :])
            nc.sync.dma_start(out=x_sb[1], in_=xTd[:, 2:4, :])
            nc.gpsimd.dma_start(out=m_sb[64:128, :], in_=mskd[64:128, :])
            nc.sync.dma_start(out=x_sb[2], in_=xTd[:, 4:6, :])
            # masked ones-feature rows (engines can't address partition 72)
            nc.gpsimd.dma_start(
                out=headT2[F2 : F2 + 1, :], in_=mskd[0:1, W : W + NQ]
            )
            nc.sync.dma_start(out=wh_sb[1], in_=whd[:, 3:6, :])
            nc.gpsimd.dma_start(out=tailT2[F2 : F2 + 1, :], in_=mskd[0:1, 0:NW])
            nc.sync.dma_start(out=wt_sb[0], in_=wtd[:, 0:3, :])
            nc.gpsimd.dma_start(out=uw2[0], in_=uw2d[:, 0:3, :])
            nc.sync.dma_start(out=wt_sb[1], in_=wtd[:, 3:6, :])
            nc.gpsimd.dma_start(out=uw2[1], in_=uw2d[:, 3:6, :])
            nc.sync.dma_start(out=uw1[0], in_=uw1d[:, 0:3, :])
            nc.gpsimd.dma_start(out=uw2[2], in_=uw2d[:, 6:9, :])
            nc.sync.dma_start(out=uw1[1], in_=uw1d[:, 3:6, :])
            nc.gpsimd.dma_start(out=uw1[2], in_=uw1d[:, 6:9, :])

            # ---- PE p-state warmup while DMAs land ----
            with tc.tile_pool(name="psj", bufs=1, space="PSUM") as psj:
                pj = psj.tile([128, 512], f32, tag="jk")
                for _ in range(6):
                    nc.tensor.matmul(
                        pj, junk[:, 0:128], junk, start=True, stop=True
                    )

            # ---- MLPs: o = gelu(W^T x + b), computed transposed ----
            bh1, bt1 = b_sb[:, 0:1], b_sb[:, 1:2]
            bh2, bt2 = b_sb[0:F2, 2:3], b_sb[0:F2, 3:4]
            with tc.tile_pool(name="psm", bufs=4, space="PSUM") as psm:
                for w_t, b1, b2, o1, o2, c0, ncols in (
                    (wh_sb, bh1, bh2, headT1, headT2, W, NQ),
                    (wt_sb, bt1, bt2, tailT1, tailT2, 0, NW),
                ):
                    for fw, f0, o, bias in ((128, 0, o1, b1), (F2, 128, o2, b2)):
                        pm = psm.tile([fw, ncols], f32, tag="pm")
                        for ht in range(6):
                            nc.tensor.matmul(
                                pm,
                                w_t[ht // 3][:, ht % 3, f0 : f0 + fw],
                                x_sb[ht // 2][:, ht % 2, c0 : c0 + ncols],
                                start=(ht == 0),
                                stop=(ht == 5),
                            )
                        nc.scalar.activation(
                            out=o[0:fw, :], in_=pm, func=gelu, bias=bias
                        )
                    nc.vector.tensor_mul(o1, o1, m_sb[0:128, c0 : c0 + ncols])
                    nc.vector.tensor_mul(
                        o2[0:F2, :], o2[0:F2, :], m_sb[0:F2, c0 : c0 + ncols]
                    )

                # ---- step A: Uh_t[j, x] = sum_i UW[t,i,j] headT[i,x] ----
                # psa nests inside psm so it gets fresh PSUM banks — the
                # first A matmuls must not WAR-wait on the MLP gelu reads
                with tc.tile_pool(name="psa", bufs=4, space="PSUM") as psa:
                    for t in range(TAGS):
                        g, tl = divmod(t, 3)
                        for jw, j0, uh in ((128, 0, uh1), (I2, 128, uh2)):
                            pa = psa.tile([jw, NQ], f32, tag="pa")
                            nc.tensor.matmul(
                                pa,
                                uw1[g][:, tl, j0 : j0 + jw],
                                headT1,
                                start=True,
                                stop=False,
                            )
                            nc.tensor.matmul(
                                pa,
                                uw2[g][:, tl, j0 : j0 + jw],
                                headT2,
                                start=False,
                                stop=True,
                            )
                            nc.any.tensor_copy(uh[:, t, :], pa)

            # ---- step B: S[m, t, x] = sum_j tailT[j, m] Uh_t[j, x] ----
            # combos: (window chunk h, query chunk xc)
            combos = {0: ((0, 0),), 1: ((1, 0), (2, 1)), 2: ((3, 1),)}
            wbq = (nc.sync, nc.gpsimd, nc.scalar)
            wbi = 0
            with tc.tile_pool(name="psb", bufs=6, space="PSUM") as psb:
                pb: dict = {}
                for h in range(3):
                    for jt, (tl_t, uh_t) in enumerate(
                        ((tailT1, uh1), (tailT2, uh2))
                    ):
                        for c, xc in combos[h]:
                            for g in range(3):
                                if jt == 0:
                                    pb[c, g] = psb.tile(
                                        [128, 3, 128],
                                        f32,
                                        tag="pb",
                                        name=f"pb{c}_{g}",
                                    )
                                nc.tensor.matmul(
                                    pb[c, g],
                                    tl_t[:, 128 * h : 128 * h + 128],
                                    uh_t[
                                        :,
                                        3 * g : 3 * g + 3,
                                        128 * xc : 128 * xc + 128,
                                    ],
                                    start=(jt == 0),
                                    stop=(jt == 1),
                                )
                    for c, xc in combos[h]:
                        for g in range(3):
                            nc.any.tensor_copy(s_cg[c][g], pb[c, g])
                            wbq[wbi % 3].dma_start(
                                out=sout[c, :, 3 * g : 3 * g + 3, :],
                                in_=s_cg[c][g],
                            )
                            wbi += 1

    nc.compile()
    return nc


def _prep_consts(Wh, bh, Wt, bt, U, Wcat, Wd):
    """Fold U/Wcat/Wd into UW[9,201,201]; arrange weights for the device."""
    import ml_dtypes

    bf16 = ml_dtypes.bfloat16

    Whp = Wcat[:, :F]  # [K, 201]
    Wtp = Wcat[:, F:]  # [K, 201]
    U2 = U.astype(np.float64).copy()
    U2[:, F - 1, :] += Wtp  # head ones-row picks up the tail term
    U2[:, :, F - 1] += Whp  # tail ones-col picks up the head term
    UW = np.einsum("kt,kij->tij", Wd.astype(np.float64), U2).astype(np.float32)
    UWi = np.ascontiguousarray(UW.transpose(1, 0, 2))  # [i, t, j]

    def tr6(w):  # [768, m] -> [128, 6, m] partition-major
        m = w.shape[1]
        return np.ascontiguousarray(
            w.reshape(6, 128, m).transpose(1, 0, 2)
        ).astype(bf16)

    return {
        "whd": tr6(Wh),
        "wtd": tr6(Wt),
        "bias4": np.ascontiguousarray(
            np.stack(
                [
                    bh[0:128],
                    bt[0:128],
                    np.pad(bh[128:BSZ], (0, 128 - F2)),
                    np.pad(bt[128:BSZ], (0, 128 - F2)),
                ],
                axis=1,
            ).astype(np.float32)
        ),
        "uw1": np.ascontiguousarray(UWi[0:128]).astype(bf16),
        "uw2": np.ascontiguousarray(UWi[128:F]).astype(bf16),
    }


def _get_nc(consts):
    key = hashlib.md5(
        b"".join(np.ascontiguousarray(v).tobytes() for v in consts.values())
    ).hexdigest()
    if _cache.get("nc_key") != key:
        _cache["nc"] = _build_nc(consts)
        _cache["nc_key"] = key
    return _cache["nc"]


def _install_ntff_hook():
    """Profiling-only (BASSK_TRACE=1): provide antenv.axon_hooks if the
    image lacks it, wired to the libaxon NTFF capture via ctypes."""
    import sys
    import types

    try:
        from antenv.axon_hooks import get_axon_ntff_profile_hook  # noqa: F401

        return
    except ImportError:
        pass
    from trn_agent_boot.trn_boot import _ntff_profile_via_ctypes

    hook = _ntff_profile_via_ctypes("/opt/axon/libaxon_pjrt.so")
    mod = types.ModuleType("antenv.axon_hooks")
    mod._hook = hook
    mod.get_axon_ntff_profile_hook = lambda: mod._hook
    mod.set_axon_ntff_profile_hook = lambda h: setattr(mod, "_hook", h)
    sys.modules["antenv.axon_hooks"] = mod


def _host_prep(state, lengths):
    """Per-core inputs: transposed state window + validity mask."""
    import ml_dtypes

    bf16 = ml_dtypes.bfloat16

    in_maps = []
    for b in range(B):
        for qi in range(N // NQ):
            q0 = qi * NQ
            lo = q0 - W
            xw = np.zeros((NW, HID), np.float32)
            s, e = max(lo, 0), min(q0 + NQ + W, N)
            xw[s - lo : e - lo] = state[b, s:e]
            pos = lo + np.arange(NW)
            mrow = ((pos >= 0) & (pos < N) & (pos < lengths[b])).astype(
                np.float32
            )
            xT = np.ascontiguousarray(xw.T)  # [768, 384]
            in_maps.append(
                {
                    "xTd": np.ascontiguousarray(
                        xT.reshape(6, 128, NW).transpose(1, 0, 2)
                    ).astype(bf16),
                    "mskd": np.ascontiguousarray(
                        np.broadcast_to(mrow[None, :], (128, NW))
                    ).astype(bf16),
                }
            )
    return in_maps


def _assemble(outs, bd):
    """outs: NCORES arrays [4, 128, TAGS, 128] -> scores [B, N, R, TAGS]."""
    scores = np.empty((B, N, R, TAGS), np.float32)
    widx = np.arange(128)[:, None] + np.arange(R)[None, :]  # [128, 129]
    xidx = np.arange(128)[:, None]
    for c, S in enumerate(outs):
        S = np.asarray(S, dtype=np.float32)  # upcast from bf16
        b, qi = divmod(c, N // NQ)
        for qc in range(2):
            # window blocks covering query chunk qc: [256 w, TAGS, 128 x]
            arr = np.concatenate([S[2 * qc], S[2 * qc + 1]], axis=0)
            g = arr[widx, :, xidx]  # [128, 129, TAGS]
            q0 = qi * NQ + qc * 128
            scores[b, q0 : q0 + 128] = g
    scores += bd.astype(np.float32)[None, None, None, :]
    return np.where(np.isfinite(scores), scores, 0.0).astype(np.float32)


def kernel(**inputs):
    state = np.asarray(inputs["state"], np.float32)
    lengths = np.asarray(inputs["lengths"]).astype(np.int64)
    Wh = np.ascontiguousarray(np.asarray(inputs["Wh"], np.float32))
    bh = np.asarray(inputs["bh"], np.float32)
    Wt = np.ascontiguousarray(np.asarray(inputs["Wt"], np.float32))
    bt = np.asarray(inputs["bt"], np.float32)
    U = np.asarray(inputs["U"], np.float32)
    Wcat = np.asarray(inputs["Wcat"], np.float32)
    Wd = np.asarray(inputs["Wd"], np.float32)
    bd = np.asarray(inputs["bd"], np.float32)

    consts = _prep_consts(Wh, bh, Wt, bt, U, Wcat, Wd)
    in_maps = _host_prep(state, lengths)
    nc = _get_nc(consts)

    if os.environ.get("BASSK_SIM"):
        from concourse.bass_interp import CoreSim

        outs = []
        for im in in_maps:
            sim = CoreSim(nc, trace=False)
            for k, v in im.items():
                sim.tensor(k)[:] = v
            sim.simulate()
            outs.append(sim.tensor("sout").copy())
    else:
        trace = bool(os.environ.get("BASSK_TRACE"))
        if trace:
            _install_ntff_hook()
        from concourse.bass_utils import run_bass_kernel_spmd

        try:
            res = run_bass_kernel_spmd(
                nc, in_maps, core_ids=list(range(NCORES)), trace=trace
            )
        except Exception:
            # transient NRT/device hiccups recover on a fresh attempt
            import time

            time.sleep(2.0)
            res = run_bass_kernel_spmd(
                nc, in_maps, core_ids=list(range(NCORES)), trace=trace
            )
        _cache["last_result"] = res
        outs = [r["sout"] for r in res.results]

    return _assemble(outs, bd)


# revision 15
# speedup vs baseline: 1.1405x; 1.0171x over previous
"""Trainium2 Bass kernel for nn_CNNNer (sparse band biaffine NER scorer).

Math collapse (everything after the GELU stage is linear):
  head = gelu(state@Wh+bh) ++ [1]          (features i = 0..200, i=200 is the 1)
  tail = gelu(state@Wt+bt) ++ [1]
  band[n,r,k] = head[n]^T U''_k tail[m],  m = n+r-64
      with U''_k = U_k + e_200 Wtp[k,:] + Whp[k,:]^T e_200^T
  scores'[n,r,t] = head_masked[n]^T UW_t tail_masked[m],
      UW_t = sum_k Wd[k,t] U''_k            (precomputed on host, [9,201,201])
  scores = scores' + bd  (host), masked-out entries = bd exactly.

Device work per core (8 cores; core = (batch b, query quarter), 256 queries,
384-position tail window). All IO in bf16 (tolerance is 2e-2):
  1. headT/tailT = gelu MLPs computed transposed ([feature, position]).
  2. step A: Uh_t[j, x] = sum_i UW[t,i,j] headT[i,x]          (9 tags)
  3. step B (tail-stationary, 6 weight loads total):
     S_t[m, x] = sum_j tailT[j, m] Uh_t[j, x] per 128-wide window chunk h
     paired with the query chunk(s) needing it: (h,xc) = (0,0),(1,0),(1,1),
     (2,1).  Band diagonals are extracted on host from the [m,t,x] blocks.

Perf structure (from trace analysis of prior versions):
  - Weights (Wh/Wt/bias/UW) are baked into the NEFF as Const tensors
    (inline_tensor), so only state-window + mask are staged per run.
    The build is cached keyed on the weight bytes; different weights
    just trigger a (seconds-long) rebuild, not wrong answers.
  - One dma_start chain can end up served by a single DMA engine
    (~20 GB/s), so sizable transfers are split into multiple chains with
    768-1536B per-partition-contiguous descriptors.
  - The scalar (Activation) queue issues NO DMAs: DIRECT2D descriptor
    generation executes on the issuing sequencer and would block the
    GELU act-table load + activations behind it.  Loads and writebacks
    go on sync (HWDGE) + gpsimd (SWDGE) queues only.
  - A short burst of junk matmuls at kernel start ramps the PE out of
    its low/mid p-state (2x clock) while the input DMAs are in flight.
"""

import hashlib
import os

import numpy as np

B, N, HID = 2, 1024, 768
BSZ = 200
W = 64
TAGS = 9
F = BSZ + 1  # 201 features incl the ones column
NQ = 256  # queries per core
NW = NQ + 2 * W  # 384 window positions per core
R = 2 * W + 1  # 129 band offsets
NCORES = 8
I2 = F - 128  # 73: second feature tile rows (i = 128..200)
F2 = BSZ - 128  # 72: second MLP output tile rows

_cache: dict = {}


def _build_nc(consts):
    import concourse.mybir as mybir
    import concourse.tile as tile
    from concourse import bacc

    dt = mybir.dt
    f32 = dt.float32
    bf16 = dt.bfloat16

    nc = bacc.Bacc(
        "TRN2", target_bir_lowering=False, debug=False, enable_asserts=False
    )
    # Per-run inputs (per-core): state window + key/query validity mask.
    xTd = nc.dram_tensor("xTd", [128, 6, NW], bf16, kind="ExternalInput").ap()
    mskd = nc.dram_tensor("mskd", [128, NW], bf16, kind="ExternalInput").ap()
    # Weights, baked into the NEFF (loaded to HBM at model-load time).
    whd = nc.inline_tensor(consts["whd"], name="whd").ap()
    wtd = nc.inline_tensor(consts["wtd"], name="wtd").ap()
    bias4 = nc.inline_tensor(consts["bias4"], name="bias4").ap()
    uw1d = nc.inline_tensor(consts["uw1"], name="uw1d").ap()
    uw2d = nc.inline_tensor(consts["uw2"], name="uw2d").ap()
    # output: 4 window-chunk/query-chunk combos of [m, t, x]
    sout = nc.dram_tensor("sout", [4, 128, TAGS, 128], bf16, kind="ExternalOutput").ap()

    gelu = {
        "gelu": mybir.ActivationFunctionType.Gelu,
        "identity": mybir.ActivationFunctionType.Identity,
    }[os.environ.get("BASSK_ACT", "gelu")]

    with tile.TileContext(nc) as tc:
        with tc.tile_pool(name="sb", bufs=1) as sb:
            # ---- SBUF tiles (split finely so loads unlock compute ASAP) ----
            x_sb = [sb.tile([128, 2, NW], bf16, name=f"x{i}") for i in range(3)]
            wh_sb = [sb.tile([128, 3, BSZ], bf16, name=f"wh{i}") for i in range(2)]
            wt_sb = [sb.tile([128, 3, BSZ], bf16, name=f"wt{i}") for i in range(2)]
            b_sb = sb.tile([128, 4], f32)
            m_sb = sb.tile([128, NW], bf16)
            uw1 = [sb.tile([128, 3, F], bf16, name=f"uw1g{g}") for g in range(3)]
            uw2 = [sb.tile([I2, 3, F], bf16, name=f"uw2g{g}") for g in range(3)]
            headT1 = sb.tile([128, NQ], bf16)
            headT2 = sb.tile([I2, NQ], bf16)
            tailT1 = sb.tile([128, NW], bf16)
            tailT2 = sb.tile([I2, NW], bf16)
            uh1 = sb.tile([128, TAGS, NQ], bf16)
            uh2 = sb.tile([I2, TAGS, NQ], bf16)
            junk = sb.tile([128, 512], bf16)
            s_cg = [
                [sb.tile([128, 3, 128], bf16, name=f"s{c}g{g}") for g in range(3)]
                for c in range(4)
            ]

            # ---- loads: sync(HWDGE) + gpsimd(SWDGE); scalar stays clean ----
            # junk memset first so the PE warmup can start ASAP
            nc.gpsimd.memset(junk, 0.0)
            nc.sync.dma_start(out=x_sb[0], in_=xTd[:, 0:2, :])
            nc.gpsimd.dma_start(out=b_sb, in_=bias4)
            nc.sync.dma_start(out=wh_sb[0], in_=whd[:, 0:3, :])
            nc.gpsimd.dma_start(out=m_sb[0:64, :], in_=mskd[0:64, :])
            nc.sync.dma_start(out=x_sb[1], in_=xTd[:, 2:4, :])
            nc.gpsimd.dma_start(out=m_sb[64:128, :], in_=mskd[64:128, :])
            nc.sync.dma_start(out=x_sb[2], in_=xTd[:, 4:6, :])
            # masked ones-feature rows (engines can't address partition 72)
            nc.gpsimd.dma_start(
                out=headT2[F2 : F2 + 1, :], in_=mskd[0:1, W : W + NQ]
            )
            nc.sync.dma_start(out=wh_sb[1], in_=whd[:, 3:6, :])
            nc.gpsimd.dma_start(out=tailT2[F2 : F2 + 1, :], in_=mskd[0:1, 0:NW])
            nc.sync.dma_start(out=wt_sb[0], in_=wtd[:, 0:3, :])
            nc.sync.dma_start(out=wt_sb[1], in_=wtd[:, 3:6, :])
            nc.gpsimd.dma_start(out=uw2[0], in_=uw2d[:, 0:3, :])
            nc.sync.dma_start(out=uw1[0], in_=uw1d[:, 0:3, :])
            nc.gpsimd.dma_start(out=uw2[1], in_=uw2d[:, 3:6, :])
            nc.sync.dma_start(out=uw1[1], in_=uw1d[:, 3:6, :])
            nc.gpsimd.dma_start(out=uw2[2], in_=uw2d[:, 6:9, :])
            nc.gpsimd.dma_start(out=uw1[2], in_=uw1d[:, 6:9, :])

            # ---- PE p-state warmup while DMAs land ----
            with tc.tile_pool(name="psj", bufs=1, space="PSUM") as psj:
                pj = psj.tile([128, 512], f32, tag="jk")
                for _ in range(6):
                    nc.tensor.matmul(
                        pj, junk[:, 0:128], junk, start=True, stop=True
                    )

            # ---- MLPs: o = gelu(W^T x + b), computed transposed ----
            bh1, bt1 = b_sb[:, 0:1], b_sb[:, 1:2]
            bh2, bt2 = b_sb[0:F2, 2:3], b_sb[0:F2, 3:4]
            with tc.tile_pool(name="psm", bufs=4, space="PSUM") as psm:
                for w_t, b1, b2, o1, o2, c0, ncols in (
                    (wh_sb, bh1, bh2, headT1, headT2, W, NQ),
                    (wt_sb, bt1, bt2, tailT1, tailT2, 0, NW),
                ):
                    for fw, f0, o, bias in ((128, 0, o1, b1), (F2, 128, o2, b2)):
                        pm = psm.tile([fw, ncols], f32, tag="pm")
                        for ht in range(6):
                            nc.tensor.matmul(
                                pm,
                                w_t[ht // 3][:, ht % 3, f0 : f0 + fw],
                                x_sb[ht // 2][:, ht % 2, c0 : c0 + ncols],
                                start=(ht == 0),
                                stop=(ht == 5),
                            )
                        nc.scalar.activation(
                            out=o[0:fw, :], in_=pm, func=gelu, bias=bias
                        )
                    nc.vector.tensor_mul(o1, o1, m_sb[0:128, c0 : c0 + ncols])
                    nc.vector.tensor_mul(
                        o2[0:F2, :], o2[0:F2, :], m_sb[0:F2, c0 : c0 + ncols]
                    )

                # ---- step A: Uh_t[j, x] = sum_i UW[t,i,j] headT[i,x] ----
                # psa nests inside psm so it gets fresh PSUM banks — the
                # first A matmuls must not WAR-wait on the MLP gelu reads
                with tc.tile_pool(name="psa", bufs=4, space="PSUM") as psa:
                    for t in range(TAGS):
                        g, tl = divmod(t, 3)
                        for jw, j0, uh in ((128, 0, uh1), (I2, 128, uh2)):
                            pa = psa.tile([jw, NQ], f32, tag="pa")
                            nc.tensor.matmul(
                                pa,
                                uw1[g][:, tl, j0 : j0 + jw],
                                headT1,
                                start=True,
                                stop=False,
                            )
                            nc.tensor.matmul(
                                pa,
                                uw2[g][:, tl, j0 : j0 + jw],
                                headT2,
                                start=False,
                                stop=True,
                            )
                            nc.any.tensor_copy(uh[:, t, :], pa)

            # ---- step B: S[m, t, x] = sum_j tailT[j, m] Uh_t[j, x] ----
            # combos: (window chunk h, query chunk xc)
            combos = {0: ((0, 0),), 1: ((1, 0), (2, 1)), 2: ((3, 1),)}
            wbq = (nc.sync, nc.gpsimd, nc.scalar)
            wbi = 0
            with tc.tile_pool(name="psb", bufs=6, space="PSUM") as psb:
                pb: dict = {}
                for h in range(3):
                    for jt, (tl_t, uh_t) in enumerate(
                        ((tailT1, uh1), (tailT2, uh2))
                    ):
                        for c, xc in combos[h]:
                            for g in range(3):
                                if jt == 0:
                                    pb[c, g] = psb.tile(
                                        [128, 3, 128],
                                        f32,
                                        tag="pb",
                                        name=f"pb{c}_{g}",
                                    )
                                nc.tensor.matmul(
                                    pb[c, g],
                                    tl_t[:, 128 * h : 128 * h + 128],
                                    uh_t[
                                        :,
                                        3 * g : 3 * g + 3,
                                        128 * xc : 128 * xc + 128,
                                    ],
                                    start=(jt == 0),
                                    stop=(jt == 1),
                                )
                    for c, xc in combos[h]:
                        for g in range(3):
                            nc.any.tensor_copy(s_cg[c][g], pb[c, g])
                            wbq[wbi % 3].dma_start(
                                out=sout[c, :, 3 * g : 3 * g + 3, :],
                                in_=s_cg[c][g],
                            )
                            wbi += 1

    nc.compile()
    return nc


def _prep_consts(Wh, bh, Wt, bt, U, Wcat, Wd):
    """Fold U/Wcat/Wd into UW[9,201,201]; arrange weights for the device."""
    import ml_dtypes

    bf16 = ml_dtypes.bfloat16

    Whp = Wcat[:, :F]  # [K, 201]
    Wtp = Wcat[:, F:]  # [K, 201]
    U2 = U.astype(np.float64).copy()
    U2[:, F - 1, :] += Wtp  # head ones-row picks up the tail term
    U2[:, :, F - 1] += Whp  # tail ones-col picks up the head term
    UW = np.einsum("kt,kij->tij", Wd.astype(np.float64), U2).astype(np.float32)
    UWi = np.ascontiguousarray(UW.transpose(1, 0, 2))  # [i, t, j]

    def tr6(w):  # [768, m] -> [128, 6, m] partition-major
        m = w.shape[1]
        return np.ascontiguousarray(
            w.reshape(6, 128, m).transpose(1, 0, 2)
        ).astype(bf16)

    return {
        "whd": tr6(Wh),
        "wtd": tr6(Wt),
        "bias4": np.ascontiguousarray(
            np.stack(
                [
                    bh[0:128],
                    bt[0:128],
                    np.pad(bh[128:BSZ], (0, 128 - F2)),
                    np.pad(bt[128:BSZ], (0, 128 - F2)),
                ],
                axis=1,
            ).astype(np.float32)
        ),
        "uw1": np.ascontiguousarray(UWi[0:128]).astype(bf16),
        "uw2": np.ascontiguousarray(UWi[128:F]).astype(bf16),
    }


def _get_nc(consts):
    key = hashlib.md5(
        b"".join(np.ascontiguousarray(v).tobytes() for v in consts.values())
    ).hexdigest()
    if _cache.get("nc_key") != key:
        _cache["nc"] = _build_nc(consts)
        _cache["nc_key"] = key
    return _cache["nc"]


def _install_ntff_hook():
    """Profiling-only (BASSK_TRACE=1): provide antenv.axon_hooks if the
    image lacks it, wired to the libaxon NTFF capture via ctypes."""
    import sys
    import types

    try:
        from antenv.axon_hooks import get_axon_ntff_profile_hook  # noqa: F401

        return
    except ImportError:
        pass
    from trn_agent_boot.trn_boot import _ntff_profile_via_ctypes

    hook = _ntff_profile_via_ctypes("/opt/axon/libaxon_pjrt.so")
    mod = types.ModuleType("antenv.axon_hooks")
    mod._hook = hook
    mod.get_axon_ntff_profile_hook = lambda: mod._hook
    mod.set_axon_ntff_profile_hook = lambda h: setattr(mod, "_hook", h)
    sys.modules["antenv.axon_hooks"] = mod


def _host_prep(state, lengths):
    """Per-core inputs: transposed state window + validity mask."""
    import ml_dtypes

    bf16 = ml_dtypes.bfloat16

    in_maps = []
    for b in range(B):
        for qi in range(N // NQ):
            q0 = qi * NQ
            lo = q0 - W
            xw = np.zeros((NW, HID), np.float32)
            s, e = max(lo, 0), min(q0 + NQ + W, N)
            xw[s - lo : e - lo] = state[b, s:e]
            pos = lo + np.arange(NW)
            mrow = ((pos >= 0) & (pos < N) & (pos < lengths[b])).astype(
                np.float32
            )
            xT = np.ascontiguousarray(xw.T)  # [768, 384]
            in_maps.append(
                {
                    "xTd": np.ascontiguousarray(
                        xT.reshape(6, 128, NW).transpose(1, 0, 2)
                    ).astype(bf16),
                    "mskd": np.ascontiguousarray(
                        np.broadcast_to(mrow[None, :], (128, NW))
                    ).astype(bf16),
                }
            )
    return in_maps


def _assemble(outs, bd):
    """outs: NCORES arrays [4, 128, TAGS, 128] -> scores [B, N, R, TAGS]."""
    scores = np.empty((B, N, R, TAGS), np.float32)
    widx = np.arange(128)[:, None] + np.arange(R)[None, :]  # [128, 129]
    xidx = np.arange(128)[:, None]
    for c, S in enumerate(outs):
        S = np.asarray(S, dtype=np.float32)  # upcast from bf16
        b, qi = divmod(c, N // NQ)
        for qc in range(2):
            # window blocks covering query chunk qc: [256 w, TAGS, 128 x]
            arr = np.concatenate([S[2 * qc], S[2 * qc + 1]], axis=0)
            g = arr[widx, :, xidx]  # [128, 129, TAGS]
            q0 = qi * NQ + qc * 128
            scores[b, q0 : q0 + 128] = g
    scores += bd.astype(np.float32)[None, None, None, :]
    return np.where(np.isfinite(scores), scores, 0.0).astype(np.float32)


def kernel(**inputs):
    state = np.asarray(inputs["state"], np.float32)
    lengths = np.asarray(inputs["lengths"]).astype(np.int64)
    Wh = np.ascontiguousarray(np.asarray(inputs["Wh"], np.float32))
    bh = np.asarray(inputs["bh"], np.float32)
    Wt = np.ascontiguousarray(np.asarray(inputs["Wt"], np.float32))
    bt = np.asarray(inputs["bt"], np.float32)
    U = np.asarray(inputs["U"], np.float32)
    Wcat = np.asarray(inputs["Wcat"], np.float32)
    Wd = np.asarray(inputs["Wd"], np.float32)
    bd = np.asarray(inputs["bd"], np.float32)

    consts = _prep_consts(Wh, bh, Wt, bt, U, Wcat, Wd)
    in_maps = _host_prep(state, lengths)
    nc = _get_nc(consts)

    if os.environ.get("BASSK_SIM"):
        from concourse.bass_interp import CoreSim

        outs = []
        for im in in_maps:
            sim = CoreSim(nc, trace=False)
            for k, v in im.items():
                sim.tensor(k)[:] = v
            sim.simulate()
            outs.append(sim.tensor("sout").copy())
    else:
        trace = bool(os.environ.get("BASSK_TRACE"))
        if trace:
            _install_ntff_hook()
        from concourse.bass_utils import run_bass_kernel_spmd

        try:
            res = run_bass_kernel_spmd(
                nc, in_maps, core_ids=list(range(NCORES)), trace=trace
            )
        except Exception:
            # transient NRT/device hiccups recover on a fresh attempt
            import time

            time.sleep(2.0)
            res = run_bass_kernel_spmd(
                nc, in_maps, core_ids=list(range(NCORES)), trace=trace
            )
        _cache["last_result"] = res
        outs = [r["sout"] for r in res.results]

    return _assemble(outs, bd)


# revision 19
# speedup vs baseline: 1.1550x; 1.0127x over previous
"""Trainium2 Bass kernel for nn_CNNNer (sparse band biaffine NER scorer).

Math collapse (everything after the GELU stage is linear):
  head = gelu(state@Wh+bh) ++ [1]          (features i = 0..200, i=200 is the 1)
  tail = gelu(state@Wt+bt) ++ [1]
  band[n,r,k] = head[n]^T U''_k tail[m],  m = n+r-64
      with U''_k = U_k + e_200 Wtp[k,:] + Whp[k,:]^T e_200^T
  scores'[n,r,t] = head_masked[n]^T UW_t tail_masked[m],
      UW_t = sum_k Wd[k,t] U''_k            (precomputed on host, [9,201,201])
  scores = scores' + bd  (host), masked-out entries = bd exactly.

Device work per core (8 cores; core = (batch b, query quarter), 256 queries,
384-position tail window). All IO in bf16 (tolerance is 2e-2):
  1. headT/tailT = gelu MLPs computed transposed ([feature, position]).
  2. step A: Uh_t[j, x] = sum_i UW[t,i,j] headT[i,x]          (9 tags)
  3. step B (tail-stationary, 6 weight loads total):
     S_t[m, x] = sum_j tailT[j, m] Uh_t[j, x] per 128-wide window chunk h
     paired with the query chunk(s) needing it: (h,xc) = (0,0),(1,0),(1,1),
     (2,1).  Band diagonals are extracted on host from the [m,t,x] blocks.

Perf structure (from trace analysis of prior versions):
  - Weights (Wh/Wt/bias/UW) are baked into the NEFF as Const tensors
    (inline_tensor), so only state-window + mask are staged per run.
    The build is cached keyed on the weight bytes; different weights
    just trigger a (seconds-long) rebuild, not wrong answers.
  - One dma_start chain can end up served by a single DMA engine
    (~20 GB/s), so sizable transfers are split into multiple chains with
    768-1536B per-partition-contiguous descriptors.
  - The scalar (Activation) queue issues NO DMAs: DIRECT2D descriptor
    generation executes on the issuing sequencer and would block the
    GELU act-table load + activations behind it.  Loads and writebacks
    go on sync (HWDGE) + gpsimd (SWDGE) queues only.
  - A short burst of junk matmuls at kernel start ramps the PE out of
    its low/mid p-state (2x clock) while the input DMAs are in flight.
"""

import hashlib
import os

import numpy as np

B, N, HID = 2, 1024, 768
BSZ = 200
W = 64
TAGS = 9
F = BSZ + 1  # 201 features incl the ones column
NQ = 256  # queries per core
NW = NQ + 2 * W  # 384 window positions per core
R = 2 * W + 1  # 129 band offsets
NCORES = 8
I2 = F - 128  # 73: second feature tile rows (i = 128..200)
F2 = BSZ - 128  # 72: second MLP output tile rows

_cache: dict = {}


def _build_nc(consts):
    import concourse.mybir as mybir
    import concourse.tile as tile
    from concourse import bacc

    dt = mybir.dt
    f32 = dt.float32
    bf16 = dt.bfloat16

    nc = bacc.Bacc(
        "TRN2", target_bir_lowering=False, debug=False, enable_asserts=False
    )
    # Per-run inputs (per-core): state window + key/query validity mask.
    xTd = nc.dram_tensor("xTd", [128, 6, NW], bf16, kind="ExternalInput").ap()
    mskd = nc.dram_tensor("mskd", [128, NW], bf16, kind="ExternalInput").ap()
    # Weights, baked into the NEFF (loaded to HBM at model-load time).
    whd = nc.inline_tensor(consts["whd"], name="whd").ap()
    wtd = nc.inline_tensor(consts["wtd"], name="wtd").ap()
    bias4 = nc.inline_tensor(consts["bias4"], name="bias4").ap()
    uw1d = nc.inline_tensor(consts["uw1"], name="uw1d").ap()
    uw2d = nc.inline_tensor(consts["uw2"], name="uw2d").ap()
    # output: 4 window-chunk/query-chunk combos of [m, t, x]
    sout = nc.dram_tensor("sout", [4, 128, TAGS, 128], bf16, kind="ExternalOutput").ap()

    gelu = {
        "gelu": mybir.ActivationFunctionType.Gelu,
        "identity": mybir.ActivationFunctionType.Identity,
    }[os.environ.get("BASSK_ACT", "gelu")]

    with tile.TileContext(nc) as tc:
        with tc.tile_pool(name="sb", bufs=1) as sb:
            # ---- SBUF tiles (split finely so loads unlock compute ASAP) ----
            x_sb = [sb.tile([128, 2, NW], bf16, name=f"x{i}") for i in range(3)]
            wh_sb = [sb.tile([128, 3, BSZ], bf16, name=f"wh{i}") for i in range(2)]
            wt_sb = [sb.tile([128, 3, BSZ], bf16, name=f"wt{i}") for i in range(2)]
            b_sb = sb.tile([128, 4], f32)
            m_sb = sb.tile([128, NW], bf16)
            uw1 = [sb.tile([128, 3, F], bf16, name=f"uw1g{g}") for g in range(3)]
            uw2 = [sb.tile([I2, 3, F], bf16, name=f"uw2g{g}") for g in range(3)]
            headT1 = sb.tile([128, NQ], bf16)
            headT2 = sb.tile([I2, NQ], bf16)
            tailT1 = sb.tile([128, NW], bf16)
            tailT2 = sb.tile([I2, NW], bf16)
            uh1 = [sb.tile([128, 3, NQ], bf16, name=f"uh1g{g}") for g in range(3)]
            uh2 = [sb.tile([I2, 3, NQ], bf16, name=f"uh2g{g}") for g in range(3)]
            junk = sb.tile([128, 512], bf16)
            s_cg = [
                [sb.tile([128, 3, 128], bf16, name=f"s{c}g{g}") for g in range(3)]
                for c in range(4)
            ]

            # ---- loads: sync(HWDGE) + gpsimd(SWDGE); scalar stays clean ----
            # junk memset first so the PE warmup can start ASAP
            nc.gpsimd.memset(junk, 0.0)
            nc.sync.dma_start(out=x_sb[0], in_=xTd[:, 0:2, :])
            nc.gpsimd.dma_start(out=b_sb, in_=bias4)
            nc.sync.dma_start(out=x_sb[1], in_=xTd[:, 2:4, :])
            nc.gpsimd.dma_start(out=m_sb[0:64, :], in_=mskd[0:64, :])
            nc.sync.dma_start(out=x_sb[2], in_=xTd[:, 4:6, :])
            nc.gpsimd.dma_start(out=m_sb[64:128, :], in_=mskd[64:128, :])
            nc.sync.dma_start(out=wh_sb[0], in_=whd[:, 0:3, :])
            # masked ones-feature rows (engines can't address partition 72)
            nc.gpsimd.dma_start(
                out=headT2[F2 : F2 + 1, :], in_=mskd[0:1, W : W + NQ]
            )
            nc.sync.dma_start(out=wh_sb[1], in_=whd[:, 3:6, :])
            nc.gpsimd.dma_start(out=tailT2[F2 : F2 + 1, :], in_=mskd[0:1, 0:NW])
            nc.sync.dma_start(out=wt_sb[0], in_=wtd[:, 0:3, :])
            nc.sync.dma_start(out=wt_sb[1], in_=wtd[:, 3:6, :])
            nc.gpsimd.dma_start(out=uw2[0], in_=uw2d[:, 0:3, :])
            nc.sync.dma_start(out=uw1[0], in_=uw1d[:, 0:3, :])
            nc.gpsimd.dma_start(out=uw2[1], in_=uw2d[:, 3:6, :])
            nc.sync.dma_start(out=uw1[1], in_=uw1d[:, 3:6, :])
            nc.gpsimd.dma_start(out=uw2[2], in_=uw2d[:, 6:9, :])
            nc.gpsimd.dma_start(out=uw1[2], in_=uw1d[:, 6:9, :])

            # ---- PE p-state warmup while DMAs land ----
            with tc.tile_pool(name="psj", bufs=1, space="PSUM") as psj:
                pj = psj.tile([128, 512], f32, tag="jk")
                for _ in range(6):
                    nc.tensor.matmul(
                        pj, junk[:, 0:128], junk, start=True, stop=True
                    )

            # ---- MLPs: o = gelu(W^T x + b), computed transposed ----
            bh1, bt1 = b_sb[:, 0:1], b_sb[:, 1:2]
            bh2, bt2 = b_sb[0:F2, 2:3], b_sb[0:F2, 3:4]
            with tc.tile_pool(name="psm", bufs=4, space="PSUM") as psm:
                for w_t, b1, b2, o1, o2, c0, ncols in (
                    (wh_sb, bh1, bh2, headT1, headT2, W, NQ),
                    (wt_sb, bt1, bt2, tailT1, tailT2, 0, NW),
                ):
                    for fw, f0, o, bias in ((128, 0, o1, b1), (F2, 128, o2, b2)):
                        pm = psm.tile([fw, ncols], f32, tag="pm")
                        for ht in range(6):
                            nc.tensor.matmul(
                                pm,
                                w_t[ht // 3][:, ht % 3, f0 : f0 + fw],
                                x_sb[ht // 2][:, ht % 2, c0 : c0 + ncols],
                                start=(ht == 0),
                                stop=(ht == 5),
                            )
                        nc.scalar.activation(
                            out=o[0:fw, :], in_=pm, func=gelu, bias=bias
                        )
                    nc.vector.tensor_mul(o1, o1, m_sb[0:128, c0 : c0 + ncols])
                    nc.vector.tensor_mul(
                        o2[0:F2, :], o2[0:F2, :], m_sb[0:F2, c0 : c0 + ncols]
                    )

                # ---- step A: Uh_t[j, x] = sum_i UW[t,i,j] headT[i,x] ----
                # psa nests inside psm so it gets fresh PSUM banks — the
                # first A matmuls must not WAR-wait on the MLP gelu reads
                with tc.tile_pool(name="psa", bufs=4, space="PSUM") as psa:
                    for t in range(TAGS):
                        g, tl = divmod(t, 3)
                        for jw, j0, uh in ((128, 0, uh1), (I2, 128, uh2)):
                            pa = psa.tile([jw, NQ], f32, tag="pa")
                            nc.tensor.matmul(
                                pa,
                                uw1[g][:, tl, j0 : j0 + jw],
                                headT1,
                                start=True,
                                stop=False,
                            )
                            nc.tensor.matmul(
                                pa,
                                uw2[g][:, tl, j0 : j0 + jw],
                                headT2,
                                start=False,
                                stop=True,
                            )
                            nc.any.tensor_copy(uh[g][:, tl, :], pa)

            # ---- step B: S[m, t, x] = sum_j tailT[j, m] Uh_t[j, x] ----
            # combos: (window chunk h, query chunk xc)
            combos = {0: ((0, 0),), 1: ((1, 0), (2, 1)), 2: ((3, 1),)}
            wbq = (nc.sync, nc.gpsimd, nc.scalar)
            wbi = 0
            with tc.tile_pool(name="psb", bufs=6, space="PSUM") as psb:
                pb: dict = {}
                for h in range(3):
                    for jt, (tl_t, uh_t) in enumerate(
                        ((tailT1, uh1), (tailT2, uh2))
                    ):
                        for c, xc in combos[h]:
                            for g in range(3):
                                if jt == 0:
                                    pb[c, g] = psb.tile(
                                        [128, 3, 128],
                                        f32,
                                        tag="pb",
                                        name=f"pb{c}_{g}",
                                    )
                                nc.tensor.matmul(
                                    pb[c, g],
                                    tl_t[:, 128 * h : 128 * h + 128],
                                    uh_t[g][
                                        :, :, 128 * xc : 128 * xc + 128
                                    ],
                                    start=(jt == 0),
                                    stop=(jt == 1),
                                )
                    for c, xc in combos[h]:
                        for g in range(3):
                            nc.any.tensor_copy(s_cg[c][g], pb[c, g])
                            wbq[wbi % 3].dma_start(
                                out=sout[c, :, 3 * g : 3 * g + 3, :],
                                in_=s_cg[c][g],
                            )
                            wbi += 1

    nc.compile()
    return nc


def _prep_consts(Wh, bh, Wt, bt, U, Wcat, Wd):
    """Fold U/Wcat/Wd into UW[9,201,201]; arrange weights for the device."""
    import ml_dtypes

    bf16 = ml_dtypes.bfloat16

    Whp = Wcat[:, :F]  # [K, 201]
    Wtp = Wcat[:, F:]  # [K, 201]
    U2 = U.astype(np.float64).copy()
    U2[:, F - 1, :] += Wtp  # head ones-row picks up the tail term
    U2[:, :, F - 1] += Whp  # tail ones-col picks up the head term
    UW = np.einsum("kt,kij->tij", Wd.astype(np.float64), U2).astype(np.float32)
    UWi = np.ascontiguousarray(UW.transpose(1, 0, 2))  # [i, t, j]

    def tr6(w):  # [768, m] -> [128, 6, m] partition-major
        m = w.shape[1]
        return np.ascontiguousarray(
            w.reshape(6, 128, m).transpose(1, 0, 2)
        ).astype(bf16)

    return {
        "whd": tr6(Wh),
        "wtd": tr6(Wt),
        "bias4": np.ascontiguousarray(
            np.stack(
                [
                    bh[0:128],
                    bt[0:128],
                    np.pad(bh[128:BSZ], (0, 128 - F2)),
                    np.pad(bt[128:BSZ], (0, 128 - F2)),
                ],
                axis=1,
            ).astype(np.float32)
        ),
        "uw1": np.ascontiguousarray(UWi[0:128]).astype(bf16),
        "uw2": np.ascontiguousarray(UWi[128:F]).astype(bf16),
    }


def _get_nc(consts):
    key = hashlib.md5(
        b"".join(np.ascontiguousarray(v).tobytes() for v in consts.values())
    ).hexdigest()
    if _cache.get("nc_key") != key:
        _cache["nc"] = _build_nc(consts)
        _cache["nc_key"] = key
    return _cache["nc"]


def _install_ntff_hook():
    """Profiling-only (BASSK_TRACE=1): provide antenv.axon_hooks if the
    image lacks it, wired to the libaxon NTFF capture via ctypes."""
    import sys
    import types

    try:
        from antenv.axon_hooks import get_axon_ntff_profile_hook  # noqa: F401

        return
    except ImportError:
        pass
    from trn_agent_boot.trn_boot import _ntff_profile_via_ctypes

    hook = _ntff_profile_via_ctypes("/opt/axon/libaxon_pjrt.so")
    mod = types.ModuleType("antenv.axon_hooks")
    mod._hook = hook
    mod.get_axon_ntff_profile_hook = lambda: mod._hook
    mod.set_axon_ntff_profile_hook = lambda h: setattr(mod, "_hook", h)
    sys.modules["antenv.axon_hooks"] = mod


def _host_prep(state, lengths):
    """Per-core inputs: transposed state window + validity mask."""
    import ml_dtypes

    bf16 = ml_dtypes.bfloat16

    in_maps = []
    for b in range(B):
        for qi in range(N // NQ):
            q0 = qi * NQ
            lo = q0 - W
            xw = np.zeros((NW, HID), np.float32)
            s, e = max(lo, 0), min(q0 + NQ + W, N)
            xw[s - lo : e - lo] = state[b, s:e]
            pos = lo + np.arange(NW)
            mrow = ((pos >= 0) & (pos < N) & (pos < lengths[b])).astype(
                np.float32
            )
            xT = np.ascontiguousarray(xw.T)  # [768, 384]
            in_maps.append(
                {
                    "xTd": np.ascontiguousarray(
                        xT.reshape(6, 128, NW).transpose(1, 0, 2)
                    ).astype(bf16),
                    "mskd": np.ascontiguousarray(
                        np.broadcast_to(mrow[None, :], (128, NW))
                    ).astype(bf16),
                }
            )
    return in_maps


def _assemble(outs, bd):
    """outs: NCORES arrays [4, 128, TAGS, 128] -> scores [B, N, R, TAGS]."""
    scores = np.empty((B, N, R, TAGS), np.float32)
    widx = np.arange(128)[:, None] + np.arange(R)[None, :]  # [128, 129]
    xidx = np.arange(128)[:, None]
    for c, S in enumerate(outs):
        S = np.asarray(S, dtype=np.float32)  # upcast from bf16
        b, qi = divmod(c, N // NQ)
        for qc in range(2):
            # window blocks covering query chunk qc: [256 w, TAGS, 128 x]
            arr = np.concatenate([S[2 * qc], S[2 * qc + 1]], axis=0)
            g = arr[widx, :, xidx]  # [128, 129, TAGS]
            q0 = qi * NQ + qc * 128
            scores[b, q0 : q0 + 128] = g
    scores += bd.astype(np.float32)[None, None, None, :]
    return np.where(np.isfinite(scores), scores, 0.0).astype(np.float32)


def kernel(**inputs):
    state = np.asarray(inputs["state"], np.float32)
    lengths = np.asarray(inputs["lengths"]).astype(np.int64)
    Wh = np.ascontiguousarray(np.asarray(inputs["Wh"], np.float32))
    bh = np.asarray(inputs["bh"], np.float32)
    Wt = np.ascontiguousarray(np.asarray(inputs["Wt"], np.float32))
    bt = np.asarray(inputs["bt"], np.float32)
    U = np.asarray(inputs["U"], np.float32)
    Wcat = np.asarray(inputs["Wcat"], np.float32)
    Wd = np.asarray(inputs["Wd"], np.float32)
    bd = np.asarray(inputs["bd"], np.float32)

    consts = _prep_consts(Wh, bh, Wt, bt, U, Wcat, Wd)
    in_maps = _host_prep(state, lengths)
    nc = _get_nc(consts)

    if os.environ.get("BASSK_SIM"):
        from concourse.bass_interp import CoreSim

        outs = []
        for im in in_maps:
            sim = CoreSim(nc, trace=False)
            for k, v in im.items():
                sim.tensor(k)[:] = v
            sim.simulate()
            outs.append(sim.tensor("sout").copy())
    else:
        trace = bool(os.environ.get("BASSK_TRACE"))
        if trace:
            _install_ntff_hook()
        from concourse.bass_utils import run_bass_kernel_spmd

        try:
            res = run_bass_kernel_spmd(
                nc, in_maps, core_ids=list(range(NCORES)), trace=trace
            )
        except Exception:
            # transient NRT/device hiccups recover on a fresh attempt
            import time

            time.sleep(2.0)
            res = run_bass_kernel_spmd(
                nc, in_maps, core_ids=list(range(NCORES)), trace=trace
            )
        _cache["last_result"] = res
        outs = [r["sout"] for r in res.results]

    return _assemble(outs, bd)
